# revision 1
# baseline (speedup 1.0000x reference)
"""Causal transformer block (B=2, S=4096, E=512, F=2048, H=8) on 8 NeuronCores.

Key insight: the reference uses a RAW reshape (B,S,E)->(B,H,S,D), so head h of
batch b only touches rows [h*512, (h+1)*512) of x[b].  The whole block therefore
decomposes into 16 fully independent 512-row chunks (QKV -> causal attention over
the (4096, 64) head view of the chunk -> out-proj -> LN -> FFN -> LN).  We run 2
chunks per core on 8 cores with zero collectives.

Per-chunk pipeline on one core (all matmuls bf16 inputs, fp32 psum):
  x (s,e) f32 -> PE-transpose -> x.T bf16
  q/k/v = (s,e') tiles via matmul(lhsT=x.T blocks, rhs=W{q,k,v}.T), bias add,
    cast bf16, bounce through HBM scratch laid out as the (4096, 64) head view
  k/q read back as (t,d) tiles (contiguous) -> PE-transpose -> k.T/q.T (d,t)
  scores S.T (t_k 128, t_q 256) = matmul(lhsT=k.T blk, rhs=q.T group), causal
    mask = -1e30 add on diagonal 128x128 blocks only, exp on ACT batched over
    1024-wide 2-bank psum tiles -> P.T bf16
  ctx (t_q, d | denom) = matmul(lhsT=P.T blk, rhs=v_aug(128,65)) accumulated in
    one psum bank per t_q block; softmax denominator is the ones-column of
    v_aug; normalize with reciprocal + tensor_scalar; bounce ctx through HBM
  ctx back as (s,e') -> PE-transpose -> ctx.T -> out-proj -> +x residual -> LN1
  FFN: h1.T = matmul(W1.T blocks, x2.T), relu+bias fused on DVE;
    y = matmul(h1.T blocks, W2.T rows); +x2 residual -> LN2 -> out

DMAs are batched (one per 0.25-1MB region, contiguous >=128B runs) and writes
issue from the gpsimd queue to keep the sync queue for loads.
"""

import sys

for _p in ("/opt/trn_rl_repo",):
    if _p not in sys.path:
        sys.path.insert(0, _p)

import numpy as np
import ml_dtypes

import concourse.bass as bass
import concourse.bacc as bacc
import concourse.mybir as mybir
import concourse.tile as tile
from concourse.alu_op_type import AluOpType
from concourse.bass_utils import run_bass_kernel_spmd

F32 = mybir.dt.float32
BF16 = mybir.dt.bfloat16
AF = mybir.ActivationFunctionType
AX = mybir.AxisListType

N_CORES = 8
E = 512          # embed dim
FF = 2048        # ffn dim
CH = 512         # rows per chunk
NCHUNK = 2       # chunks per core
T = 4096         # head-view sequence length
D = 64           # head dim
GQ = 256         # t_q group width in attention
LN_EPS = 1e-5
NEG = -1e30

_CACHE = {}
import os
USE_DVE_RSQRT = os.environ.get("K_DVE_RSQRT", "1") == "1"
USE_MASK_MUL = os.environ.get("K_MASK_MUL", "1") == "1"
USE_TTR = False  # custom-DVE ucode op crashes the exec unit on this runtime


def _emit_front(tc, c0, io, pools):
    nc = tc.nc
    sb, ps, dr = pools["sb"], pools["ps"], pools["dr"]
    w = pools["w"]

    # ---- load x chunk: one (128, 2048) f32 tile, m-block at cols 512m ----
    x4 = sb.tile([128, 4 * E], F32, tag="x", name="x", bufs=2)
    xp3 = io["xp"].rearrange("(m p) e -> p m e", p=128)  # (128, 8, 512)
    nc.sync.dma_start(
        out=x4.rearrange("p (m e) -> p m e", e=E),
        in_=xp3[:, 4 * c0 : 4 * (c0 + 1), :],
    )

    def xblk(m):
        return x4[:, E * m : E * m + E]

    # ---- x.T (e, s) bf16: 16 PE transposes, batched copies ----
    xT = []
    for k in range(4):
        tp = ps.tile([128, 512], F32, tag="med", name="med")
        for m in range(4):
            nc.tensor.transpose(
                tp[:, 128 * m : 128 * (m + 1)],
                x4[:, E * m + 128 * k : E * m + 128 * (k + 1)],
                w["identF"],
            )
        xt = sb.tile([128, CH], BF16, tag="xt", name="xt", bufs=6)
        nc.vector.tensor_copy(out=xt, in_=tp)
        xT.append(xt)

    # ---- QKV projections -> (s, e') bf16 staging -> HBM scratch ----
    q_s = dr.tile([T, D], BF16, tag="qs", name="qs", bufs=2)
    k_s = dr.tile([T, D], BF16, tag="ks", name="ks", bufs=2)
    v_s = dr.tile([T, D], BF16, tag="vs", name="vs", bufs=2)
    for wsb, bias, dst in ((w["wk"], w["bkb"], k_s), (w["wq"], w["bqb"], q_s), (w["wv"], w["bvb"], v_s)):
        st = sb.tile([128, 4 * E], BF16, tag="prj", name="prj", bufs=2)
        # (512, 512) chunk view of the (4096, 64) head-view scratch
        dst2 = dst.rearrange("(m p c) d -> p m (c d)", p=128, c=8)  # (128, 4, 512)
        for m in range(4):
            pt = ps.tile([128, E], F32, tag="med", name="med")
            for k in range(4):
                nc.tensor.matmul(
                    pt[:, 0:E],
                    lhsT=xT[k][:, 128 * m : 128 * (m + 1)],
                    rhs=wsb[:, E * k : E * (k + 1)],
                    start=(k == 0),
                    stop=(k == 3),
                )
            nc.vector.tensor_tensor(
                out=st[:, E * m : E * (m + 1)], in0=pt[:, 0:E], in1=bias, op=AluOpType.add
            )
            nc.gpsimd.dma_start(
                out=dst2[:, m, :], in_=st[:, E * m : E * (m + 1)]
            )

    # ---- k.T (d, t) build: batched (t,d) reads + PE-transpose ----
    # kT holds k.T twice: partitions 0-63 and a copy at 64-127, so scores can
    # run two K=64 row-strips of the PE array concurrently.
    k_r = k_s.rearrange("(r i p) d -> r p i d", p=128, i=4)  # (8, 128, 4, 64)
    kT = sb.tile([2 * D, T], BF16, tag="kT", name="kT", bufs=2)
    for rb in range(8):
        rt = sb.tile([128, 4 * D], BF16, tag="qkr", name="qkr", bufs=4)
        nc.sync.dma_start(out=rt.rearrange("p (i d) -> p i d", d=D), in_=k_r[rb])
        tp = ps.tile([2 * D, 512], BF16, tag="med", name="med")
        for i in range(4):
            nc.tensor.transpose(
                tp[0:D, 128 * i : 128 * (i + 1)], rt[:, D * i : D * (i + 1)], w["identB"]
            )
            nc.tensor.transpose(
                tp[D : 2 * D, 128 * i : 128 * (i + 1)],
                rt[:, D * i : D * (i + 1)],
                w["identB"],
                tile_position=(0, 64),
            )
        nc.vector.tensor_copy(out=kT[:, 512 * rb : 512 * (rb + 1)], in_=tp)

    # ---- v_aug tiles: 4 t-blocks per (128, 288) tile, ones at col 64+72i ----
    v_r = v_s.rearrange("(r i p) d -> r p i d", p=128, i=4)  # (8, 128, 4, 64)
    v_sb = []
    for rb in range(8):
        vv = sb.tile([128, 288], BF16, tag="v", name="v", bufs=18)
        vv3 = vv.rearrange("p (i c) -> p i c", c=72)
        nc.sync.dma_start(out=vv3[:, :, 0:D], in_=v_r[rb])
        nc.vector.memset(vv3[:, :, D : D + 1], 1.0)
        v_sb.append(vv)

    return {"x4": x4, "kT": kT, "v_sb": v_sb, "q_s": q_s}


def _emit_attn(tc, c0, io, pools, fr):
    nc = tc.nc
    sb, ps, dr = pools["sb"], pools["ps"], pools["dr"]
    w = pools["w"]
    kT, v_sb, q_s = fr["kT"], fr["v_sb"], fr["q_s"]

    def v_aug(b):  # (128, 65) slice for t_k block b
        return v_sb[b // 4][:, 72 * (b % 4) : 72 * (b % 4) + 65]

    # ---- attention over t_q groups of 256 (two 128-blocks per group) ----
    q_r = q_s.rearrange("(r i p) d -> r p i d", p=128, i=4)  # (8, 128, 4, 64)
    ctx_s = dr.tile([T, D], BF16, tag="cs", name="cs", bufs=2)
    ctx_w = ctx_s.rearrange("(g q p) d -> g p q d", p=128, q=2)  # (16, 128, 2, 64)
    qread = None
    for g in range(T // GQ):
        if g % 2 == 0:
            qread = sb.tile([128, 4 * D], BF16, tag="qkr", name="qkr", bufs=4)
            nc.sync.dma_start(out=qread.rearrange("p (i d) -> p i d", d=D), in_=q_r[g // 2])
        qT = sb.tile([2 * D, GQ], BF16, tag="qT", name="qT", bufs=3)
        tp = ps.tile([2 * D, 256], BF16, tag="med", name="med")
        for sub in range(2):
            i = 2 * (g % 2) + sub
            nc.tensor.transpose(
                tp[0:D, 128 * sub : 128 * (sub + 1)], qread[:, D * i : D * (i + 1)], w["identB"]
            )
            nc.tensor.transpose(
                tp[D : 2 * D, 128 * sub : 128 * (sub + 1)],
                qread[:, D * i : D * (i + 1)],
                w["identB"],
                tile_position=(0, 64),
            )
        nc.vector.tensor_copy(out=qT, in_=tp)

        # one psum bank per t_q block accumulator (cols 0:64 ctx, col 64 denom)
        cx = [ps.tile([128, 72], F32, tag="cx", name="cx") for _ in range(2)]

        nkb = 2 * g + 2  # allowed t_k blocks
        for qd in range((nkb + 3) // 4):
            kbs = list(range(4 * qd, min(4 * qd + 4, nkb)))
            pt = ps.tile([128, 1024], F32, tag="big", name="big")
            Pt = sb.tile([128, 1024], BF16, tag="P", name="P", bufs=4)
            order = (0, 2, 1, 3) if len(kbs) == 4 else range(len(kbs))
            for i in order:
                kb = kbs[i]
                if len(kbs) == 4 and i >= 2:
                    strip = slice(D, 2 * D)  # concurrent upper K=64 row-strip
                else:
                    strip = slice(0, D)
                nc.tensor.matmul(
                    pt[:, GQ * i : GQ * (i + 1)],
                    lhsT=kT[strip, 128 * kb : 128 * (kb + 1)],
                    rhs=qT[strip, :],
                    start=True,
                    stop=True,
                )
            # exp -> P.T bf16 (one instruction per quad), then zero the causal
            # triangle on diagonal blocks with a binary bf16 mask multiply
            nc.scalar.activation(
                out=Pt[:, 0 : GQ * len(kbs)], in_=pt[:, 0 : GQ * len(kbs)], func=AF.Exp
            )
            for i, kb in enumerate(kbs):
                if kb >= 2 * g:
                    j = kb - 2 * g
                    sl = Pt[:, GQ * i + 128 * j : GQ * i + 128 * (j + 1)]
                    nc.vector.tensor_tensor(out=sl, in0=sl, in1=w["binmask"], op=AluOpType.mult)
            # ctx accumulation: this P quad feeds both t_q blocks of the group
            for i, kb in enumerate(kbs):
                for qb in range(2):
                    Q = 2 * g + qb
                    if kb <= Q:
                        nc.tensor.matmul(
                            cx[qb][:, 0:65],
                            lhsT=Pt[:, GQ * i + 128 * qb : GQ * i + 128 * (qb + 1)],
                            rhs=v_aug(kb),
                            start=(kb == 0),
                            stop=(kb == Q),
                        )

        # normalize by softmax denominator (col 64), one combined write per group
        cw = sb.tile([128, 2 * D], BF16, tag="cw", name="cw", bufs=4)
        for qb in range(2):
            rc = sb.tile([128, 1], F32, tag="rc", name="rc", bufs=6)
            nc.vector.reciprocal(rc, cx[qb][:, 64:65])
            nc.vector.tensor_single_scalar(
                out=cw[:, D * qb : D * (qb + 1)], in_=cx[qb][:, 0:64], scalar=rc,
                op=AluOpType.mult,
            )
        nc.gpsimd.dma_start(out=ctx_w[g], in_=cw.rearrange("p (q d) -> p q d", d=D))
    return ctx_s


def _emit_tail(tc, c0, io, pools, fr, ctx_s):
    nc = tc.nc
    sb, ps, dr = pools["sb"], pools["ps"], pools["dr"]
    w = pools["w"]
    x4 = fr["x4"]

    def xblk(m):
        return x4[:, E * m : E * m + E]

    # ---- ctx back as (s, e'): two half reads so out-proj starts mid-attention ----
    ctx2 = ctx_s.rearrange("(m p c) d -> p m (c d)", p=128, c=8)  # (128, 4, 512)
    cT = [sb.tile([128, CH], BF16, tag="cT", name="cT", bufs=6) for _ in range(4)]
    for half in range(2):
        crh = sb.tile([128, 2 * E], BF16, tag="prj", name="prj", bufs=2)
        nc.sync.dma_start(
            out=crh.rearrange("p (m e) -> p m e", e=E),
            in_=ctx2[:, 2 * half : 2 * half + 2, :],
        )
        for j in range(4):
            tp = ps.tile([128, 256], BF16, tag="med", name="med")
            for mm in range(2):
                nc.tensor.transpose(
                    tp[:, 128 * mm : 128 * (mm + 1)],
                    crh[:, E * mm + 128 * j : E * mm + 128 * (j + 1)],
                    w["identB"],
                )
            nc.vector.tensor_copy(out=cT[j][:, 256 * half : 256 * (half + 1)], in_=tp)

    # ---- out-proj + residual + LN1 -> x2 (f32) ----
    x2_t = []
    r_list = []
    for m in range(4):
        pt = ps.tile([128, E], F32, tag="med", name="med")
        for j in range(4):
            nc.tensor.matmul(
                pt[:, 0:E],
                lhsT=cT[j][:, 128 * m : 128 * (m + 1)],
                rhs=w["wo"][:, E * j : E * (j + 1)],
                start=(j == 0),
                stop=(j == 3),
            )
        r = sb.tile([128, E], F32, tag="lnt", name="lnt", bufs=5)
        nc.vector.tensor_tensor(out=r, in0=xblk(m), in1=pt[:, 0:E], op=AluOpType.add)
        nc.vector.tensor_tensor(out=r, in0=r, in1=w["bob"], op=AluOpType.add)
        r_list.append(r)
        x2_t.append(sb.tile([128, E], F32, tag="x2", name="x2", bufs=8))
    _emit_ln_phase(tc, r_list, x2_t, w["g1b"], w["be1b"], sb, pools)

    # ---- x2.T bf16 ----
    x2T = []
    for k in range(4):
        tp = ps.tile([128, 512], F32, tag="med", name="med")
        for m in range(4):
            nc.tensor.transpose(
                tp[:, 128 * m : 128 * (m + 1)],
                x2_t[m][:, 128 * k : 128 * (k + 1)],
                w["identF"],
            )
        xt = sb.tile([128, CH], BF16, tag="xt", name="xt", bufs=6)
        nc.vector.tensor_copy(out=xt, in_=tp)
        x2T.append(xt)

    # ---- FFN1: h1.T (f, s) bf16, relu + bias fused on DVE ----
    h1 = []
    for f in range(16):
        pt = ps.tile([128, E], F32, tag="med", name="med")
        for k in range(4):
            nc.tensor.matmul(
                pt[:, 0:E],
                lhsT=w["w1"][k][:, 128 * f : 128 * (f + 1)],
                rhs=x2T[k],
                start=(k == 0),
                stop=(k == 3),
            )
        ht = sb.tile([128, E], BF16, tag="h1", name="h1", bufs=17)
        nc.vector.tensor_scalar(
            out=ht,
            in0=pt[:, 0:E],
            scalar1=w["b1c"][:, f : f + 1],
            scalar2=0.0,
            op0=AluOpType.add,
            op1=AluOpType.max,
        )
        h1.append(ht)

    # ---- FFN2 + residual + LN2 -> out (one staged write) ----
    o4 = sb.tile([128, 4 * E], F32, tag="o", name="o", bufs=2)
    r_list = []
    for m in range(4):
        pt = ps.tile([128, E], F32, tag="med", name="med")
        for f in range(16):
            nc.tensor.matmul(
                pt[:, 0:E],
                lhsT=h1[f][:, 128 * m : 128 * (m + 1)],
                rhs=w["w2"][f // 8][:, E * (f % 8) : E * (f % 8 + 1)],
                start=(f == 0),
                stop=(f == 15),
            )
        r = sb.tile([128, E], F32, tag="lnt", name="lnt", bufs=5)
        nc.vector.tensor_tensor(out=r, in0=x2_t[m], in1=pt[:, 0:E], op=AluOpType.add)
        nc.vector.tensor_tensor(out=r, in0=r, in1=w["b2b"], op=AluOpType.add)
        r_list.append(r)
    _emit_ln_phase(
        tc, r_list, [o4[:, E * m : E * (m + 1)] for m in range(4)],
        w["g2b"], w["be2b"], sb, pools,
    )
    out3 = io["out"].rearrange("(m p) e -> p m e", p=128)
    nc.gpsimd.dma_start(
        out=out3[:, 4 * c0 : 4 * (c0 + 1), :],
        in_=o4.rearrange("p (m e) -> p m e", e=E),
    )


def _emit_ln_phase(tc, rs_list, out_list, gb, beb, sb, pools):
    """Batched row-wise LN over four (128, 512) tiles sharing one rsqrt.

    out = (r - mean(r)) * rsqrt(var(r) + eps) * g + be
    rsqrt runs on DVE (Quake seed + 3 Newton steps) on a (128, 4) stats tile,
    keeping ScalarE free for exp (no activation-table thrashing).
    """
    nc = tc.nc
    n = len(rs_list)
    stats = sb.tile([128, 4], F32, tag="st4", name="st4", bufs=4)
    xcs = []
    for m, r in enumerate(rs_list):
        rsum = sb.tile([128, 1], F32, tag="st", name="st", bufs=16)
        nc.vector.reduce_sum(rsum, r, axis=AX.X)
        mu = sb.tile([128, 1], F32, tag="st", name="st", bufs=16)
        nc.vector.tensor_single_scalar(out=mu, in_=rsum, scalar=1.0 / E, op=AluOpType.mult)
        xc = sb.tile([128, E], F32, tag="lnt", name="lnt", bufs=5)
        nc.vector.tensor_single_scalar(out=xc, in_=r, scalar=mu, op=AluOpType.subtract)
        sq = sb.tile([128, E], F32, tag="sq", name="sq", bufs=2)
        nc.gpsimd.tensor_tensor(out=sq, in0=xc, in1=xc, op=AluOpType.mult)
        nc.vector.reduce_sum(stats[:, m : m + 1], sq, axis=AX.X)
        xcs.append(xc)
    if not USE_DVE_RSQRT:
        for m, (xc, out) in enumerate(zip(xcs, out_list)):
            std = sb.tile([128, 1], F32, tag="st", name="st", bufs=16)
            nc.scalar.activation(out=std, in_=stats[:, m : m + 1], func=AF.Sqrt,
                                 bias=pools["w"]["epsc"], scale=1.0 / E)
            rstd = sb.tile([128, 1], F32, tag="st", name="st", bufs=16)
            nc.vector.reciprocal(rstd, std)
            nc.vector.scalar_tensor_tensor(
                out=out, in0=xc, scalar=rstd, in1=gb,
                op0=AluOpType.mult, op1=AluOpType.mult,
            )
            nc.vector.tensor_tensor(out=out, in0=out, in1=beb, op=AluOpType.add)
        return
    # v = ss/E + eps; rstd = rsqrt(v) via bit trick + 3 Newton iterations
    v = sb.tile([128, 4], F32, tag="st4", name="st4", bufs=4)
    nc.vector.tensor_scalar(
        out=v[:, 0:n], in0=stats[:, 0:n], scalar1=1.0 / E, scalar2=LN_EPS,
        op0=AluOpType.mult, op1=AluOpType.add,
    )
    w = pools["w"]
    y = sb.tile([128, 4], F32, tag="st4", name="st4", bufs=4)
    yi = y.bitcast(mybir.dt.int32)
    nc.vector.tensor_single_scalar(
        out=yi[:, 0:n], in_=v.bitcast(mybir.dt.int32)[:, 0:n], scalar=1,
        op=AluOpType.arith_shift_right,
    )
    nc.vector.tensor_tensor(
        out=yi[:, 0:n], in0=w["magic"][:, 0:n], in1=yi[:, 0:n], op=AluOpType.subtract
    )
    for _ in range(3):
        a = sb.tile([128, 4], F32, tag="st4", name="st4", bufs=4)
        nc.vector.tensor_tensor(out=a[:, 0:n], in0=y[:, 0:n], in1=y[:, 0:n], op=AluOpType.mult)
        nc.vector.tensor_tensor(out=a[:, 0:n], in0=a[:, 0:n], in1=v[:, 0:n], op=AluOpType.mult)
        nc.vector.tensor_scalar(
            out=a[:, 0:n], in0=a[:, 0:n], scalar1=-0.5, scalar2=1.5,
            op0=AluOpType.mult, op1=AluOpType.add,
        )
        nc.vector.tensor_tensor(out=y[:, 0:n], in0=y[:, 0:n], in1=a[:, 0:n], op=AluOpType.mult)
    for m, (xc, out) in enumerate(zip(xcs, out_list)):
        nc.vector.scalar_tensor_tensor(
            out=out, in0=xc, scalar=y[:, m : m + 1], in1=gb,
            op0=AluOpType.mult, op1=AluOpType.mult,
        )
        nc.gpsimd.tensor_tensor(out=out, in0=out, in1=beb, op=AluOpType.add)


def _build(repeat=1):
    nc = bacc.Bacc("TRN2", target_bir_lowering=False, debug=False)

    io = {}
    io["xp"] = nc.dram_tensor("xp", [NCHUNK * CH, E], F32, kind="ExternalInput").ap()
    for nm, shp, dt in (
        ("wqT", [E, E], BF16),
        ("wkT", [E, E], BF16),
        ("wvT", [E, E], BF16),
        ("woT", [E, E], BF16),
        ("w1T", [E, FF], BF16),
        ("w2T", [FF, E], BF16),
        ("bqb", [128, E], F32),
        ("bkb", [128, E], F32),
        ("bvb", [128, E], F32),
        ("bob", [128, E], F32),
        ("b2b", [128, E], F32),
        ("b1c", [FF, 1], F32),
        ("g1b", [128, E], F32),
        ("be1b", [128, E], F32),
        ("g2b", [128, E], F32),
        ("be2b", [128, E], F32),
        ("binmask", [128, 128], BF16),
        ("magic", [128, 4], F32),
        ("identF", [128, 128], F32),
        ("identB", [128, 128], BF16),
    ):
        io[nm] = nc.dram_tensor(nm, shp, dt, kind="ExternalInput").ap()
    io["out"] = nc.dram_tensor("out", [NCHUNK * CH, E], F32, kind="ExternalOutput").ap()

    with tile.TileContext(nc) as tc:
        with (
            tc.tile_pool(name="sb", bufs=2) as sb,
            tc.tile_pool(name="ps", space="PSUM", bufs=2) as ps,
            tc.tile_pool(name="dr", space="DRAM", bufs=2) as dr,
        ):
            w = {}
            # square weights: one (128, 2048) tile each, k-block at cols 512k
            for nm, key in (("wqT", "wq"), ("wkT", "wk"), ("wvT", "wv"), ("woT", "wo")):
                wt = sb.tile([128, 4 * E], BF16, tag=nm, name=nm, bufs=1)
                nc.sync.dma_start(
                    out=wt.rearrange("p (k e) -> p k e", e=E),
                    in_=io[nm].rearrange("(k p) e -> p k e", p=128),
                )
                w[key] = wt
            def _load_ffn_weights():
                # deferred: emitted after the QKV fronts so these 4MB don't
                # contend with the critical x/Wqkv loads at kernel start
                w["w1"] = []
                for k in range(4):
                    wt = sb.tile([128, FF], BF16, tag="w1", name="w1", bufs=4)
                    nc.sync.dma_start(out=wt, in_=io["w1T"][128 * k : 128 * (k + 1), :])
                    w["w1"].append(wt)
                w2r = io["w2T"].rearrange("(i j p) e -> i p j e", p=128, j=8)
                w["w2"] = []
                for i in range(2):
                    wt = sb.tile([128, 8 * E], BF16, tag="w2", name="w2", bufs=2)
                    nc.sync.dma_start(out=wt.rearrange("p (j e) -> p j e", e=E), in_=w2r[i])
                    w["w2"].append(wt)
            # broadcast biases / LN consts / masks
            for nm in ("bqb", "bkb", "bvb", "bob", "b2b", "g1b", "be1b", "g2b", "be2b", "binmask", "identF", "identB"):
                dt = BF16 if nm in ("identB", "binmask") else F32
                shp = [128, 128] if nm in ("binmask", "identF", "identB") else [128, E]
                wt = sb.tile(shp, dt, tag=nm, name=nm, bufs=1)
                nc.sync.dma_start(out=wt, in_=io[nm])
                w[nm] = wt
            # rsqrt magic constant (0x5f3759df) as int32 tile
            magic = sb.tile([128, 4], F32, tag="magic", name="magic", bufs=1)
            nc.sync.dma_start(out=magic, in_=io["magic"])
            w["magic"] = magic.bitcast(mybir.dt.int32)
            epsc = sb.tile([128, 1], F32, tag="epsc", name="epsc", bufs=1)
            nc.vector.memset(epsc, LN_EPS)
            w["epsc"] = epsc
            # b1 as (128, 16) per-partition columns
            wt = sb.tile([128, 16], F32, tag="b1c", name="b1c", bufs=1)
            nc.sync.dma_start(
                out=wt.rearrange("p (f o) -> p f o", o=1),
                in_=io["b1c"].rearrange("(f p) o -> p f o", p=128),
            )
            w["b1c"] = wt

            pools = {"sb": sb, "ps": ps, "dr": dr, "w": w}
            for _rep in range(repeat):
                frs = [_emit_front(tc, c0, io, pools) for c0 in range(NCHUNK)]
                if "w1" not in w:
                    _load_ffn_weights()
                ctxs = [_emit_attn(tc, c0, io, pools, frs[c0]) for c0 in range(NCHUNK)]
                for c0 in range(NCHUNK):
                    _emit_tail(tc, c0, io, pools, frs[c0], ctxs[c0])

    nc.compile()
    return nc


def _host_inputs(x, Wq, bq, Wk, bk, Wv, bv, Wo, bo, W1, b1, W2, b2, g1, be1, g2, be2):
    bf = ml_dtypes.bfloat16
    f32 = np.float32

    def bT(wm):
        return np.ascontiguousarray(np.asarray(wm, f32).T).astype(bf)

    def bc(v):
        return np.ascontiguousarray(np.tile(np.asarray(v, f32)[None, :], (128, 1)))

    ii, jj = np.indices((128, 128))
    binmask = np.where(jj >= ii, 1.0, 0.0).astype(ml_dtypes.bfloat16)

    base = {
        "wqT": bT(Wq), "wkT": bT(Wk), "wvT": bT(Wv), "woT": bT(Wo),
        "w1T": bT(W1), "w2T": bT(W2),
        "bqb": bc(bq), "bkb": bc(bk), "bvb": bc(bv), "bob": bc(bo), "b2b": bc(b2),
        "b1c": np.asarray(b1, f32).reshape(FF, 1).copy(),
        "g1b": bc(g1), "be1b": bc(be1), "g2b": bc(g2), "be2b": bc(be2),
        "binmask": binmask,
        "magic": np.full((128, 4), np.float32(np.frombuffer(np.uint32(0x5F3759DF).tobytes(), np.float32)[0])),
        "identF": np.eye(128, dtype=f32),
        "identB": np.eye(128, dtype=f32).astype(bf),
    }
    xf = np.asarray(x, f32).reshape(N_CORES * NCHUNK * CH, E)
    in_maps = []
    for c in range(N_CORES):
        m = dict(base)
        m["xp"] = np.ascontiguousarray(xf[c * NCHUNK * CH : (c + 1) * NCHUNK * CH, :])
        in_maps.append(m)
    return in_maps


def get_nc():
    if "nc" not in _CACHE:
        _CACHE["nc"] = _build()
    return _CACHE["nc"]


def run(trace=False, **inputs):
    nc = get_nc()
    in_maps = _host_inputs(**inputs)
    res = run_bass_kernel_spmd(nc, in_maps, core_ids=list(range(N_CORES)), trace=trace)
    B, S = 2, 4096
    out = np.concatenate([res.results[c]["out"] for c in range(N_CORES)], axis=0)
    return out.reshape(B, S, E).astype(np.float32), res


def kernel(**inputs):
    out, _ = run(trace=False, **inputs)
    return out


def bench_nc(nc, in_maps, n_iter=10):
    """Steady-state timing of an arbitrary prebuilt nc via persistent jit."""
    import time
    import jax
    from jax.sharding import Mesh, PartitionSpec
    from jax.experimental.shard_map import shard_map
    from concourse.bass2jax import _bass_exec_p, install_neuronx_cc_hook, partition_id_tensor

    install_neuronx_cc_hook()
    partition_name = nc.partition_id_tensor.name if nc.partition_id_tensor else None
    in_names, out_names, out_avals, zero_outs = [], [], [], []
    for alloc in nc.m.functions[0].allocations:
        if not isinstance(alloc, mybir.MemoryLocationSet):
            continue
        name = alloc.memorylocations[0].name
        if alloc.kind == "ExternalInput":
            if name != partition_name:
                in_names.append(name)
        elif alloc.kind == "ExternalOutput":
            out_names.append(name)
            shape = tuple(alloc.tensor_shape)
            dtype = mybir.dt.np(alloc.dtype)
            import jax as _jax
            out_avals.append(_jax.core.ShapedArray(shape, dtype))
            zero_outs.append(np.zeros(shape, dtype))
    n_params = len(in_names)
    all_in_names = in_names + out_names
    if partition_name is not None:
        all_in_names = all_in_names + [partition_name]

    def _body(*args):
        operands = list(args)
        if partition_name is not None:
            operands.append(partition_id_tensor())
        outs = _bass_exec_p.bind(
            *operands,
            out_avals=tuple(out_avals),
            in_names=tuple(all_in_names),
            out_names=tuple(out_names),
            lowering_input_output_aliases=(),
            sim_require_finite=True,
            sim_require_nnan=True,
            nc=nc,
        )
        return tuple(outs)

    devices = jax.devices()[:N_CORES]
    mesh = Mesh(np.asarray(devices), ("core",))
    in_specs = (PartitionSpec("core"),) * (n_params + len(out_names))
    out_specs = (PartitionSpec("core"),) * len(out_names)
    sharded = jax.jit(
        shard_map(_body, mesh=mesh, in_specs=in_specs, out_specs=out_specs, check_rep=False),
        keep_unused=True,
    )
    per_core = [[np.asarray(m[name]) for name in in_names] for m in in_maps]
    concat_in = [
        np.concatenate([per_core[c][i] for c in range(N_CORES)], axis=0)
        for i in range(n_params)
    ]
    concat_zeros = [np.zeros((N_CORES * z.shape[0], *z.shape[1:]), z.dtype) for z in zero_outs]
    sharding = jax.sharding.NamedSharding(mesh, PartitionSpec("core"))
    dev_in = [jax.device_put(a, sharding) for a in concat_in + concat_zeros]

    out = sharded(*dev_in)
    jax.block_until_ready(out)
    times = []
    for _ in range(n_iter):
        t0 = time.perf_counter()
        out = sharded(*dev_in)
        jax.block_until_ready(out)
        times.append(time.perf_counter() - t0)
    return times, out


def bench(n_iter=10, **inputs):
    nc = get_nc()
    in_maps = _host_inputs(**inputs)
    times, out = bench_nc(nc, in_maps, n_iter)
    arr = np.asarray(out[0]).reshape(N_CORES, NCHUNK * CH, E).reshape(-1, E)
    return times, arr.reshape(2, 4096, E)


def bench_repeat(R=8, n_iter=10, **inputs):
    """Estimate per-body device time from the R-repeat slope."""
    in_maps = _host_inputs(**inputs)
    ncR = _build(repeat=R)
    tR, out = bench_nc(ncR, in_maps, n_iter)
    t1, _ = bench_nc(get_nc(), in_maps, n_iter)
    body_ns = (min(tR) - min(t1)) / (R - 1) * 1e9
    return body_ns, t1, tR, out


def bench_burst(nc, in_maps, bursts=(1, 4), n_rep=6):
    """Per-call device time from burst slope: time(k calls, one final block).
    Returns ns per call (dispatch overhead cancels in the slope)."""
    import time
    import jax
    res = {}
    sharded, dev_in = _make_exec(nc, in_maps)
    out = sharded(*dev_in); jax.block_until_ready(out)  # warm
    for k in bursts:
        best = None
        for _ in range(n_rep):
            t0 = time.perf_counter()
            for _ in range(k):
                out = sharded(*dev_in)
            jax.block_until_ready(out)
            dt = time.perf_counter() - t0
            best = dt if best is None else min(best, dt)
        res[k] = best
    ks = sorted(res)
    slope = (res[ks[-1]] - res[ks[0]]) / (ks[-1] - ks[0])
    return slope * 1e9, res


def _make_exec(nc, in_maps):
    import jax
    from jax.sharding import Mesh, PartitionSpec
    from jax.experimental.shard_map import shard_map
    from concourse.bass2jax import _bass_exec_p, install_neuronx_cc_hook, partition_id_tensor

    install_neuronx_cc_hook()
    partition_name = nc.partition_id_tensor.name if nc.partition_id_tensor else None
    in_names, out_names, out_avals, zero_outs = [], [], [], []
    for alloc in nc.m.functions[0].allocations:
        if not isinstance(alloc, mybir.MemoryLocationSet):
            continue
        name = alloc.memorylocations[0].name
        if alloc.kind == "ExternalInput":
            if name != partition_name:
                in_names.append(name)
        elif alloc.kind == "ExternalOutput":
            out_names.append(name)
            shape = tuple(alloc.tensor_shape)
            dtype = mybir.dt.np(alloc.dtype)
            out_avals.append(jax.core.ShapedArray(shape, dtype))
            zero_outs.append(np.zeros(shape, dtype))
    n_params = len(in_names)
    all_in_names = in_names + out_names
    if partition_name is not None:
        all_in_names = all_in_names + [partition_name]

    def _body(*args):
        operands = list(args)
        if partition_name is not None:
            operands.append(partition_id_tensor())
        outs = _bass_exec_p.bind(
            *operands,
            out_avals=tuple(out_avals),
            in_names=tuple(all_in_names),
            out_names=tuple(out_names),
            lowering_input_output_aliases=(),
            sim_require_finite=True,
            sim_require_nnan=True,
            nc=nc,
        )
        return tuple(outs)

    devices = jax.devices()[:N_CORES]
    mesh = Mesh(np.asarray(devices), ("core",))
    in_specs = (PartitionSpec("core"),) * (n_params + len(out_names))
    out_specs = (PartitionSpec("core"),) * len(out_names)
    sharded = jax.jit(
        shard_map(_body, mesh=mesh, in_specs=in_specs, out_specs=out_specs, check_rep=False),
        keep_unused=True,
    )
    per_core = [[np.asarray(m[name]) for name in in_names] for m in in_maps]
    concat_in = [
        np.concatenate([per_core[c][i] for c in range(N_CORES)], axis=0)
        for i in range(n_params)
    ]
    concat_zeros = [np.zeros((N_CORES * z.shape[0], *z.shape[1:]), z.dtype) for z in zero_outs]
    sharding = jax.sharding.NamedSharding(mesh, PartitionSpec("core"))
    dev_in = [jax.device_put(a, sharding) for a in concat_in + concat_zeros]
    return sharded, dev_in



# revision 2
# speedup vs baseline: 226.7382x; 226.7382x over previous
"""Causal transformer block (B=2, S=4096, E=512, F=2048, H=8) on 8 NeuronCores.

Key insight: the reference uses a RAW reshape (B,S,E)->(B,H,S,D), so head h of
batch b only touches rows [h*512, (h+1)*512) of x[b].  The whole block therefore
decomposes into 16 fully independent 512-row chunks (QKV -> causal attention over
the (4096, 64) head view of the chunk -> out-proj -> LN -> FFN -> LN).  We run 2
chunks per core on 8 cores with zero collectives.

Per-chunk pipeline on one core (all matmuls bf16 inputs, fp32 psum):
  x (s,e) f32 -> PE-transpose -> x.T bf16
  q/k/v = (s,e') tiles via matmul(lhsT=x.T blocks, rhs=W{q,k,v}.T), bias add,
    cast bf16, bounce through HBM scratch laid out as the (4096, 64) head view
  k/q read back as (t,d) tiles (contiguous) -> PE-transpose -> k.T/q.T (d,t)
  scores S.T (t_k 128, t_q 256) = matmul(lhsT=k.T blk, rhs=q.T group), causal
    mask = -1e30 add on diagonal 128x128 blocks only, exp on ACT batched over
    1024-wide 2-bank psum tiles -> P.T bf16
  ctx (t_q, d | denom) = matmul(lhsT=P.T blk, rhs=v_aug(128,65)) accumulated in
    one psum bank per t_q block; softmax denominator is the ones-column of
    v_aug; normalize with reciprocal + tensor_scalar; bounce ctx through HBM
  ctx back as (s,e') -> PE-transpose -> ctx.T -> out-proj -> +x residual -> LN1
  FFN: h1.T = matmul(W1.T blocks, x2.T), relu+bias fused on DVE;
    y = matmul(h1.T blocks, W2.T rows); +x2 residual -> LN2 -> out

DMAs are batched (one per 0.25-1MB region, contiguous >=128B runs) and writes
issue from the gpsimd queue to keep the sync queue for loads.
"""

import sys

for _p in ("/opt/trn_rl_repo",):
    if _p not in sys.path:
        sys.path.insert(0, _p)

import numpy as np
import ml_dtypes

import concourse.bass as bass
import concourse.bacc as bacc
import concourse.mybir as mybir
import concourse.tile as tile
from concourse.alu_op_type import AluOpType
from concourse.bass_utils import run_bass_kernel_spmd

F32 = mybir.dt.float32
BF16 = mybir.dt.bfloat16
AF = mybir.ActivationFunctionType
AX = mybir.AxisListType

N_CORES = 8
E = 512          # embed dim
FF = 2048        # ffn dim
CH = 512         # rows per chunk
NCHUNK = 2       # chunks per core
T = 4096         # head-view sequence length
D = 64           # head dim
GQ = 256         # t_q group width in attention
LN_EPS = 1e-5
NEG = -1e30

_CACHE = {}
import os
USE_DVE_RSQRT = os.environ.get("K_DVE_RSQRT", "1") == "1"
USE_MASK_MUL = os.environ.get("K_MASK_MUL", "1") == "1"
USE_TTR = False  # custom-DVE ucode op crashes the exec unit on this runtime


def _emit_front(tc, c0, io, pools):
    nc = tc.nc
    sb, ps, dr = pools["sb"], pools["ps"], pools["dr"]
    w = pools["w"]

    # ---- load x chunk: one (128, 2048) f32 tile, m-block at cols 512m ----
    x4 = sb.tile([128, 4 * E], F32, tag="x", name="x", bufs=2)
    xp3 = io["xp"].rearrange("(m p) e -> p m e", p=128)  # (128, 8, 512)
    nc.sync.dma_start(
        out=x4.rearrange("p (m e) -> p m e", e=E),
        in_=xp3[:, 4 * c0 : 4 * (c0 + 1), :],
    )

    def xblk(m):
        return x4[:, E * m : E * m + E]

    # ---- x.T (e, s) fp8 single tile (k-subtile dim for DoubleRow) ----
    xT = sb.tile([128, 4, CH], F8, tag="xt", name="xt", bufs=2)
    for k in range(4):
        tp = ps.tile([128, 512], F32, tag="med", name="med")
        for m in range(4):
            nc.tensor.transpose(
                tp[:, 128 * m : 128 * (m + 1)],
                x4[:, E * m + 128 * k : E * m + 128 * (k + 1)],
                w["identF"],
            )
        nc.vector.tensor_copy(out=xT[:, k, :], in_=tp)

    # ---- QKV projections (fp8 DoubleRow, weights pre-scaled x16) ----
    # q/k: out = psum/16 + b; v: out = 16*psum + 256*b  (v staged x256 so the
    # fp8 ctx cast downstream stays in normal range; undone after out-proj)
    q_s = dr.tile([T, D], BF16, tag="qs", name="qs", bufs=2)
    k_s = dr.tile([T, D], BF16, tag="ks", name="ks", bufs=2)
    v_s = dr.tile([T, D], BF16, tag="vs", name="vs", bufs=2)
    for wsb, bias, dst, scl, nw in (
        (w["wk"], w["bkb"], k_s, 1.0 / 16, 2),
        (w["wq"], w["bqb"], q_s, 1.0 / 16, 1),
        (w["wv"], w["bvb"], v_s, 16.0, 1),
    ):
        st = sb.tile([128, 4 * E], BF16, tag="prj", name="prj", bufs=2)
        w3 = wsb.rearrange("p (k e) -> p k e", e=E)  # (128, 4, 512) fp8
        # (512, 512) chunk view of the (4096, 64) head-view scratch
        dst2 = dst.rearrange("(m p c) d -> p m (c d)", p=128, c=8)  # (128, 4, 512)
        for m in range(4):
            pt = ps.tile([128, E], F32, tag="med", name="med")
            for k in range(2):
                nc.tensor.matmul(
                    pt[:, 0:E],
                    lhsT=xT[:, 2 * k : 2 * k + 2, 128 * m : 128 * (m + 1)],
                    rhs=w3[:, 2 * k : 2 * k + 2, :],
                    start=(k == 0),
                    stop=(k == 1),
                    perf_mode=mybir.MatmulPerfMode.DoubleRow,
                )
            nc.vector.scalar_tensor_tensor(
                out=st[:, E * m : E * (m + 1)], in0=pt[:, 0:E], scalar=scl, in1=bias,
                op0=AluOpType.mult, op1=AluOpType.add,
            )
            # batched staging writes: k in halves (kT build reads it first),
            # q/v in one DMA each
            if (m + 1) % (4 // nw) == 0:
                lo = E * (m + 1 - 4 // nw)
                nc.gpsimd.dma_start(
                    out=dst2[:, m + 1 - 4 // nw : m + 1, :],
                    in_=st[:, lo : E * (m + 1)].rearrange("p (m e) -> p m e", e=E),
                )

    # ---- k.T (d, t) build: batched (t,d) reads + PE-transpose ----
    # kT holds k.T twice: partitions 0-63 and a copy at 64-127, so scores can
    # run two K=64 row-strips of the PE array concurrently.
    k_r = k_s.rearrange("(r i p) d -> r p i d", p=128, i=4)  # (8, 128, 4, 64)
    kT = sb.tile([2 * D, T], BF16, tag="kT", name="kT", bufs=2)
    for rb in range(8):
        rt = sb.tile([128, 4 * D], BF16, tag="qkr", name="qkr", bufs=4)
        nc.sync.dma_start(out=rt.rearrange("p (i d) -> p i d", d=D), in_=k_r[rb])
        tp = ps.tile([2 * D, 512], BF16, tag="med", name="med")
        for i in range(4):
            nc.tensor.transpose(
                tp[0:D, 128 * i : 128 * (i + 1)], rt[:, D * i : D * (i + 1)], w["identB"]
            )
            nc.tensor.transpose(
                tp[D : 2 * D, 128 * i : 128 * (i + 1)],
                rt[:, D * i : D * (i + 1)],
                w["identB"],
                tile_position=(0, 64),
            )
        nc.vector.tensor_copy(out=kT[:, 512 * rb : 512 * (rb + 1)], in_=tp)

    # ---- v_aug tiles: 4 t-blocks per (128, 288) tile, ones at col 64+72i ----
    v_r = v_s.rearrange("(r i p) d -> r p i d", p=128, i=4)  # (8, 128, 4, 64)
    v_sb = []
    for rb in range(8):
        vv = sb.tile([128, 288], BF16, tag="v", name="v", bufs=18)
        vv3 = vv.rearrange("p (i c) -> p i c", c=72)
        nc.sync.dma_start(out=vv3[:, :, 0:D], in_=v_r[rb])
        nc.vector.memset(vv3[:, :, D : D + 1], 1.0)
        v_sb.append(vv)

    return {"x4": x4, "kT": kT, "v_sb": v_sb, "q_s": q_s}


def _emit_attn(tc, c0, io, pools, fr):
    nc = tc.nc
    sb, ps, dr = pools["sb"], pools["ps"], pools["dr"]
    w = pools["w"]
    kT, v_sb, q_s = fr["kT"], fr["v_sb"], fr["q_s"]

    def v_aug(b):  # (128, 65) slice for t_k block b
        return v_sb[b // 4][:, 72 * (b % 4) : 72 * (b % 4) + 65]

    # ---- attention over t_q groups of 256 (two 128-blocks per group) ----
    q_r = q_s.rearrange("(r i p) d -> r p i d", p=128, i=4)  # (8, 128, 4, 64)
    ctx_s = dr.tile([T, D], BF16, tag="cs", name="cs", bufs=2)
    ctx_w = ctx_s.rearrange("(g q p) d -> g p q d", p=128, q=2)  # (16, 128, 2, 64)
    qread = None
    for g in range(T // GQ):
        if g % 2 == 0:
            qread = sb.tile([128, 4 * D], BF16, tag="qkr", name="qkr", bufs=4)
            nc.sync.dma_start(out=qread.rearrange("p (i d) -> p i d", d=D), in_=q_r[g // 2])
        qT = sb.tile([2 * D, GQ], BF16, tag="qT", name="qT", bufs=3)
        tp = ps.tile([2 * D, 256], BF16, tag="med", name="med")
        for sub in range(2):
            i = 2 * (g % 2) + sub
            nc.tensor.transpose(
                tp[0:D, 128 * sub : 128 * (sub + 1)], qread[:, D * i : D * (i + 1)], w["identB"]
            )
            nc.tensor.transpose(
                tp[D : 2 * D, 128 * sub : 128 * (sub + 1)],
                qread[:, D * i : D * (i + 1)],
                w["identB"],
                tile_position=(0, 64),
            )
        nc.vector.tensor_copy(out=qT, in_=tp)

        # one psum bank per t_q block accumulator (cols 0:64 ctx, col 64 denom)
        cx = [ps.tile([128, 72], F32, tag="cx", name="cx") for _ in range(2)]

        nkb = 2 * g + 2  # allowed t_k blocks
        for qd in range((nkb + 3) // 4):
            kbs = list(range(4 * qd, min(4 * qd + 4, nkb)))
            pt = ps.tile([128, 1024], F32, tag="big", name="big")
            Pt = sb.tile([128, 1024], BF16, tag="P", name="P", bufs=4)
            order = (0, 2, 1, 3) if len(kbs) == 4 else range(len(kbs))
            for i in order:
                kb = kbs[i]
                if len(kbs) == 4 and i >= 2:
                    strip = slice(D, 2 * D)  # concurrent upper K=64 row-strip
                else:
                    strip = slice(0, D)
                nc.tensor.matmul(
                    pt[:, GQ * i : GQ * (i + 1)],
                    lhsT=kT[strip, 128 * kb : 128 * (kb + 1)],
                    rhs=qT[strip, :],
                    start=True,
                    stop=True,
                )
            # exp -> P.T bf16 (one instruction per quad), then zero the causal
            # triangle on diagonal blocks with a binary bf16 mask multiply
            nc.scalar.activation(
                out=Pt[:, 0 : GQ * len(kbs)], in_=pt[:, 0 : GQ * len(kbs)], func=AF.Exp
            )
            for i, kb in enumerate(kbs):
                if kb >= 2 * g:
                    j = kb - 2 * g
                    sl = Pt[:, GQ * i + 128 * j : GQ * i + 128 * (j + 1)]
                    nc.vector.tensor_tensor(out=sl, in0=sl, in1=w["binmask"], op=AluOpType.mult)
            # ctx accumulation: this P quad feeds both t_q blocks of the group
            for i, kb in enumerate(kbs):
                for qb in range(2):
                    Q = 2 * g + qb
                    if kb <= Q:
                        nc.tensor.matmul(
                            cx[qb][:, 0:65],
                            lhsT=Pt[:, GQ * i + 128 * qb : GQ * i + 128 * (qb + 1)],
                            rhs=v_aug(kb),
                            start=(kb == 0),
                            stop=(kb == Q),
                        )

        # normalize by softmax denominator (col 64), one combined write per group
        cw = sb.tile([128, 2 * D], BF16, tag="cw", name="cw", bufs=4)
        for qb in range(2):
            rc = sb.tile([128, 1], F32, tag="rc", name="rc", bufs=6)
            nc.vector.reciprocal(rc, cx[qb][:, 64:65])
            nc.vector.tensor_single_scalar(
                out=cw[:, D * qb : D * (qb + 1)], in_=cx[qb][:, 0:64], scalar=rc,
                op=AluOpType.mult,
            )
        nc.gpsimd.dma_start(out=ctx_w[g], in_=cw.rearrange("p (q d) -> p q d", d=D))
    return ctx_s


def _emit_tail(tc, c0, io, pools, fr, ctx_s):
    nc = tc.nc
    sb, ps, dr = pools["sb"], pools["ps"], pools["dr"]
    w = pools["w"]
    x4 = fr["x4"]

    def xblk(m):
        return x4[:, E * m : E * m + E]

    # ---- ctx back as (s, e'): two half reads so out-proj starts mid-attention ----
    ctx2 = ctx_s.rearrange("(m p c) d -> p m (c d)", p=128, c=8)  # (128, 4, 512)
    cT = [sb.tile([128, CH], BF16, tag="cT", name="cT", bufs=6) for _ in range(4)]
    for half in range(2):
        crh = sb.tile([128, 2 * E], BF16, tag="prj", name="prj", bufs=2)
        nc.sync.dma_start(
            out=crh.rearrange("p (m e) -> p m e", e=E),
            in_=ctx2[:, 2 * half : 2 * half + 2, :],
        )
        for j in range(4):
            tp = ps.tile([128, 256], BF16, tag="med", name="med")
            for mm in range(2):
                nc.tensor.transpose(
                    tp[:, 128 * mm : 128 * (mm + 1)],
                    crh[:, E * mm + 128 * j : E * mm + 128 * (j + 1)],
                    w["identB"],
                )
            nc.vector.tensor_copy(out=cT[j][:, 256 * half : 256 * (half + 1)], in_=tp)

    # ---- out-proj + residual + LN1 -> x2 (f32) ----
    x2_t = []
    r_list = []
    for m in range(4):
        pt = ps.tile([128, E], F32, tag="med", name="med")
        for j in range(4):
            nc.tensor.matmul(
                pt[:, 0:E],
                lhsT=cT[j][:, 128 * m : 128 * (m + 1)],
                rhs=w["wo"][:, E * j : E * (j + 1)],
                start=(j == 0),
                stop=(j == 3),
            )
        r = sb.tile([128, E], F32, tag="lnt", name="lnt", bufs=5)
        nc.vector.tensor_tensor(out=r, in0=xblk(m), in1=pt[:, 0:E], op=AluOpType.add)
        nc.vector.tensor_tensor(out=r, in0=r, in1=w["bob"], op=AluOpType.add)
        r_list.append(r)
        x2_t.append(sb.tile([128, E], F32, tag="x2", name="x2", bufs=8))
    _emit_ln_phase(tc, r_list, x2_t, w["g1b"], w["be1b"], sb, pools)

    # ---- x2.T bf16 ----
    x2T = []
    for k in range(4):
        tp = ps.tile([128, 512], F32, tag="med", name="med")
        for m in range(4):
            nc.tensor.transpose(
                tp[:, 128 * m : 128 * (m + 1)],
                x2_t[m][:, 128 * k : 128 * (k + 1)],
                w["identF"],
            )
        xt = sb.tile([128, CH], BF16, tag="xt", name="xt", bufs=6)
        nc.vector.tensor_copy(out=xt, in_=tp)
        x2T.append(xt)

    # ---- FFN1: h1.T (f, s) bf16, relu + bias fused on DVE ----
    h1 = []
    for f in range(16):
        pt = ps.tile([128, E], F32, tag="med", name="med")
        for k in range(4):
            nc.tensor.matmul(
                pt[:, 0:E],
                lhsT=w["w1"][k][:, 128 * f : 128 * (f + 1)],
                rhs=x2T[k],
                start=(k == 0),
                stop=(k == 3),
            )
        ht = sb.tile([128, E], BF16, tag="h1", name="h1", bufs=17)
        nc.vector.tensor_scalar(
            out=ht,
            in0=pt[:, 0:E],
            scalar1=w["b1c"][:, f : f + 1],
            scalar2=0.0,
            op0=AluOpType.add,
            op1=AluOpType.max,
        )
        h1.append(ht)

    # ---- FFN2 + residual + LN2 -> out (one staged write) ----
    o4 = sb.tile([128, 4 * E], F32, tag="o", name="o", bufs=2)
    r_list = []
    for m in range(4):
        pt = ps.tile([128, E], F32, tag="med", name="med")
        for f in range(16):
            nc.tensor.matmul(
                pt[:, 0:E],
                lhsT=h1[f][:, 128 * m : 128 * (m + 1)],
                rhs=w["w2"][f // 8][:, E * (f % 8) : E * (f % 8 + 1)],
                start=(f == 0),
                stop=(f == 15),
            )
        r = sb.tile([128, E], F32, tag="lnt", name="lnt", bufs=5)
        nc.vector.tensor_tensor(out=r, in0=x2_t[m], in1=pt[:, 0:E], op=AluOpType.add)
        nc.vector.tensor_tensor(out=r, in0=r, in1=w["b2b"], op=AluOpType.add)
        r_list.append(r)
    _emit_ln_phase(
        tc, r_list, [o4[:, E * m : E * (m + 1)] for m in range(4)],
        w["g2b"], w["be2b"], sb, pools,
    )
    out3 = io["out"].rearrange("(m p) e -> p m e", p=128)
    nc.gpsimd.dma_start(
        out=out3[:, 4 * c0 : 4 * (c0 + 1), :],
        in_=o4.rearrange("p (m e) -> p m e", e=E),
    )


def _emit_ln_phase(tc, rs_list, out_list, gb, beb, sb, pools):
    """Batched row-wise LN over four (128, 512) tiles sharing one rsqrt.

    out = (r - mean(r)) * rsqrt(var(r) + eps) * g + be
    rsqrt runs on DVE (Quake seed + 3 Newton steps) on a (128, 4) stats tile,
    keeping ScalarE free for exp (no activation-table thrashing).
    """
    nc = tc.nc
    n = len(rs_list)
    stats = sb.tile([128, 4], F32, tag="st4", name="st4", bufs=4)
    xcs = []
    for m, r in enumerate(rs_list):
        rsum = sb.tile([128, 1], F32, tag="st", name="st", bufs=16)
        nc.vector.reduce_sum(rsum, r, axis=AX.X)
        mu = sb.tile([128, 1], F32, tag="st", name="st", bufs=16)
        nc.vector.tensor_single_scalar(out=mu, in_=rsum, scalar=1.0 / E, op=AluOpType.mult)
        xc = sb.tile([128, E], F32, tag="lnt", name="lnt", bufs=5)
        nc.vector.tensor_single_scalar(out=xc, in_=r, scalar=mu, op=AluOpType.subtract)
        sq = sb.tile([128, E], F32, tag="sq", name="sq", bufs=2)
        nc.gpsimd.tensor_tensor(out=sq, in0=xc, in1=xc, op=AluOpType.mult)
        nc.vector.reduce_sum(stats[:, m : m + 1], sq, axis=AX.X)
        xcs.append(xc)
    if not USE_DVE_RSQRT:
        for m, (xc, out) in enumerate(zip(xcs, out_list)):
            std = sb.tile([128, 1], F32, tag="st", name="st", bufs=16)
            nc.scalar.activation(out=std, in_=stats[:, m : m + 1], func=AF.Sqrt,
                                 bias=pools["w"]["epsc"], scale=1.0 / E)
            rstd = sb.tile([128, 1], F32, tag="st", name="st", bufs=16)
            nc.vector.reciprocal(rstd, std)
            nc.vector.scalar_tensor_tensor(
                out=out, in0=xc, scalar=rstd, in1=gb,
                op0=AluOpType.mult, op1=AluOpType.mult,
            )
            nc.vector.tensor_tensor(out=out, in0=out, in1=beb, op=AluOpType.add)
        return
    # v = ss/E + eps; rstd = rsqrt(v) via bit trick + 3 Newton iterations
    v = sb.tile([128, 4], F32, tag="st4", name="st4", bufs=4)
    nc.vector.tensor_scalar(
        out=v[:, 0:n], in0=stats[:, 0:n], scalar1=1.0 / E, scalar2=LN_EPS,
        op0=AluOpType.mult, op1=AluOpType.add,
    )
    w = pools["w"]
    y = sb.tile([128, 4], F32, tag="st4", name="st4", bufs=4)
    yi = y.bitcast(mybir.dt.int32)
    nc.vector.tensor_single_scalar(
        out=yi[:, 0:n], in_=v.bitcast(mybir.dt.int32)[:, 0:n], scalar=1,
        op=AluOpType.arith_shift_right,
    )
    nc.vector.tensor_tensor(
        out=yi[:, 0:n], in0=w["magic"][:, 0:n], in1=yi[:, 0:n], op=AluOpType.subtract
    )
    for _ in range(3):
        a = sb.tile([128, 4], F32, tag="st4", name="st4", bufs=4)
        nc.vector.tensor_tensor(out=a[:, 0:n], in0=y[:, 0:n], in1=y[:, 0:n], op=AluOpType.mult)
        nc.vector.tensor_tensor(out=a[:, 0:n], in0=a[:, 0:n], in1=v[:, 0:n], op=AluOpType.mult)
        nc.vector.tensor_scalar(
            out=a[:, 0:n], in0=a[:, 0:n], scalar1=-0.5, scalar2=1.5,
            op0=AluOpType.mult, op1=AluOpType.add,
        )
        nc.vector.tensor_tensor(out=y[:, 0:n], in0=y[:, 0:n], in1=a[:, 0:n], op=AluOpType.mult)
    for m, (xc, out) in enumerate(zip(xcs, out_list)):
        nc.vector.scalar_tensor_tensor(
            out=out, in0=xc, scalar=y[:, m : m + 1], in1=gb,
            op0=AluOpType.mult, op1=AluOpType.mult,
        )
        nc.gpsimd.tensor_tensor(out=out, in0=out, in1=beb, op=AluOpType.add)


def _build(repeat=1):
    nc = bacc.Bacc("TRN2", target_bir_lowering=False, debug=False)

    io = {}
    io["xp"] = nc.dram_tensor("xp", [NCHUNK * CH, E], F32, kind="ExternalInput").ap()
    for nm, shp, dt in (
        ("wqT", [E, E], BF16),
        ("wkT", [E, E], BF16),
        ("wvT", [E, E], BF16),
        ("woT", [E, E], BF16),
        ("w1T", [E, FF], BF16),
        ("w2T", [FF, E], BF16),
        ("bqb", [128, E], F32),
        ("bkb", [128, E], F32),
        ("bvb", [128, E], F32),
        ("bob", [128, E], F32),
        ("b2b", [128, E], F32),
        ("b1c", [FF, 1], F32),
        ("g1b", [128, E], F32),
        ("be1b", [128, E], F32),
        ("g2b", [128, E], F32),
        ("be2b", [128, E], F32),
        ("binmask", [128, 128], BF16),
        ("magic", [128, 4], F32),
        ("identF", [128, 128], F32),
        ("identB", [128, 128], BF16),
    ):
        io[nm] = nc.dram_tensor(nm, shp, dt, kind="ExternalInput").ap()
    io["out"] = nc.dram_tensor("out", [NCHUNK * CH, E], F32, kind="ExternalOutput").ap()

    with tile.TileContext(nc) as tc:
        with (
            tc.tile_pool(name="sb", bufs=2) as sb,
            tc.tile_pool(name="ps", space="PSUM", bufs=2) as ps,
            tc.tile_pool(name="dr", space="DRAM", bufs=2) as dr,
        ):
            w = {}
            # square weights: one (128, 2048) tile each, k-block at cols 512k
            for nm, key in (("wqT", "wq"), ("wkT", "wk"), ("wvT", "wv"), ("woT", "wo")):
                wt = sb.tile([128, 4 * E], BF16, tag=nm, name=nm, bufs=1)
                nc.sync.dma_start(
                    out=wt.rearrange("p (k e) -> p k e", e=E),
                    in_=io[nm].rearrange("(k p) e -> p k e", p=128),
                )
                w[key] = wt
            def _load_ffn_weights():
                # deferred: emitted after the QKV fronts so these 4MB don't
                # contend with the critical x/Wqkv loads at kernel start
                w["w1"] = []
                for k in range(4):
                    wt = sb.tile([128, FF], BF16, tag="w1", name="w1", bufs=4)
                    nc.sync.dma_start(out=wt, in_=io["w1T"][128 * k : 128 * (k + 1), :])
                    w["w1"].append(wt)
                w2r = io["w2T"].rearrange("(i j p) e -> i p j e", p=128, j=8)
                w["w2"] = []
                for i in range(2):
                    wt = sb.tile([128, 8 * E], BF16, tag="w2", name="w2", bufs=2)
                    nc.sync.dma_start(out=wt.rearrange("p (j e) -> p j e", e=E), in_=w2r[i])
                    w["w2"].append(wt)
            # broadcast biases / LN consts / masks
            for nm in ("bqb", "bkb", "bvb", "bob", "b2b", "g1b", "be1b", "g2b", "be2b", "binmask", "identF", "identB"):
                dt = BF16 if nm in ("identB", "binmask") else F32
                shp = [128, 128] if nm in ("binmask", "identF", "identB") else [128, E]
                wt = sb.tile(shp, dt, tag=nm, name=nm, bufs=1)
                nc.sync.dma_start(out=wt, in_=io[nm])
                w[nm] = wt
            # rsqrt magic constant (0x5f3759df) as int32 tile
            magic = sb.tile([128, 4], F32, tag="magic", name="magic", bufs=1)
            nc.sync.dma_start(out=magic, in_=io["magic"])
            w["magic"] = magic.bitcast(mybir.dt.int32)
            epsc = sb.tile([128, 1], F32, tag="epsc", name="epsc", bufs=1)
            nc.vector.memset(epsc, LN_EPS)
            w["epsc"] = epsc
            # b1 as (128, 16) per-partition columns
            wt = sb.tile([128, 16], F32, tag="b1c", name="b1c", bufs=1)
            nc.sync.dma_start(
                out=wt.rearrange("p (f o) -> p f o", o=1),
                in_=io["b1c"].rearrange("(f p) o -> p f o", p=128),
            )
            w["b1c"] = wt

            pools = {"sb": sb, "ps": ps, "dr": dr, "w": w}
            for _rep in range(repeat):
                frs = [_emit_front(tc, c0, io, pools) for c0 in range(NCHUNK)]
                if "w1" not in w:
                    _load_ffn_weights()
                ctxs = [_emit_attn(tc, c0, io, pools, frs[c0]) for c0 in range(NCHUNK)]
                for c0 in range(NCHUNK):
                    _emit_tail(tc, c0, io, pools, frs[c0], ctxs[c0])

    nc.compile()
    return nc


def _host_inputs(x, Wq, bq, Wk, bk, Wv, bv, Wo, bo, W1, b1, W2, b2, g1, be1, g2, be2):
    bf = ml_dtypes.bfloat16
    f32 = np.float32

    def bT(wm):
        return np.ascontiguousarray(np.asarray(wm, f32).T).astype(bf)

    def bc(v):
        return np.ascontiguousarray(np.tile(np.asarray(v, f32)[None, :], (128, 1)))

    ii, jj = np.indices((128, 128))
    binmask = np.where(jj >= ii, 1.0, 0.0).astype(ml_dtypes.bfloat16)

    base = {
        "wqT": bT(Wq), "wkT": bT(Wk), "wvT": bT(Wv), "woT": bT(Wo),
        "w1T": bT(W1), "w2T": bT(W2),
        "bqb": bc(bq), "bkb": bc(bk), "bvb": bc(bv), "bob": bc(bo), "b2b": bc(b2),
        "b1c": np.asarray(b1, f32).reshape(FF, 1).copy(),
        "g1b": bc(g1), "be1b": bc(be1), "g2b": bc(g2), "be2b": bc(be2),
        "binmask": binmask,
        "magic": np.full((128, 4), np.float32(np.frombuffer(np.uint32(0x5F3759DF).tobytes(), np.float32)[0])),
        "identF": np.eye(128, dtype=f32),
        "identB": np.eye(128, dtype=f32).astype(bf),
    }
    xf = np.asarray(x, f32).reshape(N_CORES * NCHUNK * CH, E)
    in_maps = []
    for c in range(N_CORES):
        m = dict(base)
        m["xp"] = np.ascontiguousarray(xf[c * NCHUNK * CH : (c + 1) * NCHUNK * CH, :])
        in_maps.append(m)
    return in_maps


def get_nc():
    if "nc" not in _CACHE:
        _CACHE["nc"] = _build()
    return _CACHE["nc"]


def run(trace=False, **inputs):
    nc = get_nc()
    in_maps = _host_inputs(**inputs)
    res = run_bass_kernel_spmd(nc, in_maps, core_ids=list(range(N_CORES)), trace=trace)
    B, S = 2, 4096
    out = np.concatenate([res.results[c]["out"] for c in range(N_CORES)], axis=0)
    return out.reshape(B, S, E).astype(np.float32), res


def kernel(**inputs):
    out, _ = run(trace=False, **inputs)
    return out


def bench_nc(nc, in_maps, n_iter=10):
    """Steady-state timing of an arbitrary prebuilt nc via persistent jit."""
    import time
    import jax
    from jax.sharding import Mesh, PartitionSpec
    from jax.experimental.shard_map import shard_map
    from concourse.bass2jax import _bass_exec_p, install_neuronx_cc_hook, partition_id_tensor

    install_neuronx_cc_hook()
    partition_name = nc.partition_id_tensor.name if nc.partition_id_tensor else None
    in_names, out_names, out_avals, zero_outs = [], [], [], []
    for alloc in nc.m.functions[0].allocations:
        if not isinstance(alloc, mybir.MemoryLocationSet):
            continue
        name = alloc.memorylocations[0].name
        if alloc.kind == "ExternalInput":
            if name != partition_name:
                in_names.append(name)
        elif alloc.kind == "ExternalOutput":
            out_names.append(name)
            shape = tuple(alloc.tensor_shape)
            dtype = mybir.dt.np(alloc.dtype)
            import jax as _jax
            out_avals.append(_jax.core.ShapedArray(shape, dtype))
            zero_outs.append(np.zeros(shape, dtype))
    n_params = len(in_names)
    all_in_names = in_names + out_names
    if partition_name is not None:
        all_in_names = all_in_names + [partition_name]

    def _body(*args):
        operands = list(args)
        if partition_name is not None:
            operands.append(partition_id_tensor())
        outs = _bass_exec_p.bind(
            *operands,
            out_avals=tuple(out_avals),
            in_names=tuple(all_in_names),
            out_names=tuple(out_names),
            lowering_input_output_aliases=(),
            sim_require_finite=True,
            sim_require_nnan=True,
            nc=nc,
        )
        return tuple(outs)

    devices = jax.devices()[:N_CORES]
    mesh = Mesh(np.asarray(devices), ("core",))
    in_specs = (PartitionSpec("core"),) * (n_params + len(out_names))
    out_specs = (PartitionSpec("core"),) * len(out_names)
    sharded = jax.jit(
        shard_map(_body, mesh=mesh, in_specs=in_specs, out_specs=out_specs, check_rep=False),
        keep_unused=True,
    )
    per_core = [[np.asarray(m[name]) for name in in_names] for m in in_maps]
    concat_in = [
        np.concatenate([per_core[c][i] for c in range(N_CORES)], axis=0)
        for i in range(n_params)
    ]
    concat_zeros = [np.zeros((N_CORES * z.shape[0], *z.shape[1:]), z.dtype) for z in zero_outs]
    sharding = jax.sharding.NamedSharding(mesh, PartitionSpec("core"))
    dev_in = [jax.device_put(a, sharding) for a in concat_in + concat_zeros]

    out = sharded(*dev_in)
    jax.block_until_ready(out)
    times = []
    for _ in range(n_iter):
        t0 = time.perf_counter()
        out = sharded(*dev_in)
        jax.block_until_ready(out)
        times.append(time.perf_counter() - t0)
    return times, out


def bench(n_iter=10, **inputs):
    nc = get_nc()
    in_maps = _host_inputs(**inputs)
    times, out = bench_nc(nc, in_maps, n_iter)
    arr = np.asarray(out[0]).reshape(N_CORES, NCHUNK * CH, E).reshape(-1, E)
    return times, arr.reshape(2, 4096, E)


def bench_repeat(R=8, n_iter=10, **inputs):
    """Estimate per-body device time from the R-repeat slope."""
    in_maps = _host_inputs(**inputs)
    ncR = _build(repeat=R)
    tR, out = bench_nc(ncR, in_maps, n_iter)
    t1, _ = bench_nc(get_nc(), in_maps, n_iter)
    body_ns = (min(tR) - min(t1)) / (R - 1) * 1e9
    return body_ns, t1, tR, out


def bench_burst(nc, in_maps, bursts=(1, 4), n_rep=6):
    """Per-call device time from burst slope: time(k calls, one final block).
    Returns ns per call (dispatch overhead cancels in the slope)."""
    import time
    import jax
    res = {}
    sharded, dev_in = _make_exec(nc, in_maps)
    out = sharded(*dev_in); jax.block_until_ready(out)  # warm
    for k in bursts:
        best = None
        for _ in range(n_rep):
            t0 = time.perf_counter()
            for _ in range(k):
                out = sharded(*dev_in)
            jax.block_until_ready(out)
            dt = time.perf_counter() - t0
            best = dt if best is None else min(best, dt)
        res[k] = best
    ks = sorted(res)
    slope = (res[ks[-1]] - res[ks[0]]) / (ks[-1] - ks[0])
    return slope * 1e9, res


def _make_exec(nc, in_maps):
    import jax
    from jax.sharding import Mesh, PartitionSpec
    from jax.experimental.shard_map import shard_map
    from concourse.bass2jax import _bass_exec_p, install_neuronx_cc_hook, partition_id_tensor

    install_neuronx_cc_hook()
    partition_name = nc.partition_id_tensor.name if nc.partition_id_tensor else None
    in_names, out_names, out_avals, zero_outs = [], [], [], []
    for alloc in nc.m.functions[0].allocations:
        if not isinstance(alloc, mybir.MemoryLocationSet):
            continue
        name = alloc.memorylocations[0].name
        if alloc.kind == "ExternalInput":
            if name != partition_name:
                in_names.append(name)
        elif alloc.kind == "ExternalOutput":
            out_names.append(name)
            shape = tuple(alloc.tensor_shape)
            dtype = mybir.dt.np(alloc.dtype)
            out_avals.append(jax.core.ShapedArray(shape, dtype))
            zero_outs.append(np.zeros(shape, dtype))
    n_params = len(in_names)
    all_in_names = in_names + out_names
    if partition_name is not None:
        all_in_names = all_in_names + [partition_name]

    def _body(*args):
        operands = list(args)
        if partition_name is not None:
            operands.append(partition_id_tensor())
        outs = _bass_exec_p.bind(
            *operands,
            out_avals=tuple(out_avals),
            in_names=tuple(all_in_names),
            out_names=tuple(out_names),
            lowering_input_output_aliases=(),
            sim_require_finite=True,
            sim_require_nnan=True,
            nc=nc,
        )
        return tuple(outs)

    devices = jax.devices()[:N_CORES]
    mesh = Mesh(np.asarray(devices), ("core",))
    in_specs = (PartitionSpec("core"),) * (n_params + len(out_names))
    out_specs = (PartitionSpec("core"),) * len(out_names)
    sharded = jax.jit(
        shard_map(_body, mesh=mesh, in_specs=in_specs, out_specs=out_specs, check_rep=False),
        keep_unused=True,
    )
    per_core = [[np.asarray(m[name]) for name in in_names] for m in in_maps]
    concat_in = [
        np.concatenate([per_core[c][i] for c in range(N_CORES)], axis=0)
        for i in range(n_params)
    ]
    concat_zeros = [np.zeros((N_CORES * z.shape[0], *z.shape[1:]), z.dtype) for z in zero_outs]
    sharding = jax.sharding.NamedSharding(mesh, PartitionSpec("core"))
    dev_in = [jax.device_put(a, sharding) for a in concat_in + concat_zeros]
    return sharded, dev_in

